# revision 16
# baseline (speedup 1.0000x reference)
"""Contrastive (SimCLR-style) loss on 8 Trainium2 NeuronCores.

Math (matches the reference exactly):
  P = concat(projection1, projection2)            # [8192, 256]
  sim = cos_sim(P_i, P_j); diag masked to -1e9; logits = sim / 0.5
  labels = arange(2B)  -> picks the masked diagonal, so
  loss = -mean_i( logp_ii ),  logp_ii = f32(-2e9 - lse_i),
  lse_i = log(sum_{j != i} exp(2*sim_ij))

Distribution: data-parallel over the 8192 rows; every core holds the full
bf16 P^T as the moving matmul operand and its own RAW 1024-column slice as
the stationary operand (no on-chip transpose).  Per core:
  - global column norms: DVE squares of the P^T tiles + a ones-stationary
    TensorE matmul reducing over partitions, a small PSUM->SBUF->SBUF
    compaction, Newton rsqrt on a [32, 64] slice per 2048-column group,
  - Q^T = P^T * (1/n_j) via a DRAM-bounced partition broadcast (bf16, DVE),
  - the row-side 1/n_i rides the ScalarE activation's per-partition scale
    (exp(scale_i * psum), scale_i = 2/n_i) so it never touches operands,
  - matmul row-block x all 8192 columns (bf16, fp32 PSUM, 2048-col groups),
  - exp streamed through ScalarE with fused row-sum accumulation,
  - rowsum - e^2 (diagonal of the normalized Gram is cos=1 -> exp(2)), log.

Scheduling: DMA completion follows issue order, so the group-0-critical
loads go first on the cheap sync triggers and the bulk follows on gpsimd;
later groups' prep is fenced with tile_wait_until so the greedy tile
scheduler cannot wedge it into the group-0 critical chain; the ScalarE Exp
table is preloaded by a dummy activation at t=0.
Host applies the reference's fp32 arithmetic for the final mean.
"""

import sys

for _p in ("/opt/trn_rl_repo", "/root/.axon_site/_ro/trn_rl_repo"):
    if _p not in sys.path:
        sys.path.append(_p)

import numpy as np

import concourse.bacc as bacc
import concourse.tile as tile
from concourse import mybir
from concourse import bass_utils

F32 = mybir.dt.float32
BF16 = mybir.dt.bfloat16
I32 = mybir.dt.int32
AF = mybir.ActivationFunctionType
ALU = mybir.AluOpType

N_CORES = 8
B = 8192          # total rows (2 * batch)
D = 256           # projection dim
BLK = B // N_CORES        # 1024 rows per core
M_TILES = BLK // 128      # 8 row tiles per core
N_COLS = 512              # matmul free dim (one PSUM bank)
GROUP = 2048              # ACT exp batch (4 PSUM banks) = one column group
N_GROUPS = B // GROUP     # 4
N_PER_GROUP = GROUP // N_COLS  # 4
UO = BLK // 128           # 8: row tiles in the own-block stats load
RSQRT_MAGIC = 0x5F3759DF
E2 = 7.38905609893065     # exp(2): the masked diagonal's exp term


def _newton_rsqrt(nc, pool, out_rn, s, tag):
    """out_rn = 1/sqrt(s), entirely on VectorE (fp32).

    Quake-style bit seed + 2 Newton iterations (~5e-6 rel err).  Keeps
    ScalarE free for exp and avoids sqrt<->exp table reloads.
    """
    p, w = s.shape
    ibits = pool.tile([p, w], I32, name="ibits", tag=f"rsqi_{tag}", bufs=1)
    nc.vector.tensor_scalar(
        out=ibits, in0=s.bitcast(I32), scalar1=1, scalar2=None,
        op0=ALU.arith_shift_right,
    )
    nc.vector.tensor_scalar(
        out=ibits, in0=ibits, scalar1=-1, scalar2=RSQRT_MAGIC,
        op0=ALU.mult, op1=ALU.add,
    )
    y = ibits.bitcast(F32)
    t1 = pool.tile([p, w], F32, name="t1", tag=f"rsqt_{tag}", bufs=1)
    for _ in range(2):
        nc.vector.tensor_mul(t1, y, y)
        nc.vector.tensor_mul(t1, t1, s)
        nc.vector.tensor_scalar(
            out=t1, in0=t1, scalar1=-0.5, scalar2=1.5,
            op0=ALU.mult, op1=ALU.add,
        )
        nc.vector.tensor_mul(y, y, t1)
    nc.vector.tensor_copy(out_rn, y)


def _emit(tc, pt, pblkt, pblk, lse_out):
    nc = tc.nc

    persist = tc.alloc_tile_pool(name="persist", bufs=1)
    pin = tc.alloc_tile_pool(name="pin", bufs=2)
    work = tc.alloc_tile_pool(name="work", bufs=2)
    dram = tc.alloc_tile_pool(name="dram", bufs=1, space="DRAM")
    epool = tc.alloc_tile_pool(name="epool", bufs=2)
    psum_pool = tc.alloc_tile_pool(name="psum", bufs=2, space="PSUM")

    # Persistent tensors
    ptk = [persist.tile([128, B], BF16, tag=f"pt{k}", name=f"pt{k}")
           for k in range(2)]
    qtk = [persist.tile([128, B], BF16, tag=f"qt{k}", name=f"qt{k}")
           for k in range(2)]
    pblkt_t = [persist.tile([128, BLK], BF16, tag=f"pbt{k}", name=f"pbt{k}")
               for k in range(2)]
    sqt = [persist.tile([128, GROUP], BF16, tag=f"sqt{k}", name=f"sqt{k}")
           for k in range(2)]
    ones = persist.tile([128, 128], BF16, tag="ones", name="ones")
    nsq = persist.tile([32, 4 * 64], F32, tag="nsq", name="nsq")
    rn_b16 = persist.tile([32, 4 * 64], BF16, tag="rn_b16", name="rn_b16")
    nblk = persist.tile([128, UO], BF16, tag="nblk", name="nblk")
    nblk_f = persist.tile([128, UO], F32, tag="nblk_f", name="nblk_f")
    scv = persist.tile([128, UO], F32, tag="scv", name="scv")
    sums = persist.tile([128, N_GROUPS * M_TILES], F32, tag="sums", name="sums")
    rowsum = persist.tile([128, M_TILES], F32, tag="rowsum", name="rowsum")
    lse = persist.tile([128, M_TILES], F32, tag="lse", name="lse")
    warm = persist.tile([1, 2], F32, tag="warm", name="warm")
    dram_rn = dram.tile([B], BF16, tag="dram_rn", name="dram_rn")
    dram_nsq = dram.tile([N_GROUPS * GROUP], F32, tag="dram_nsq", name="dram_nsq")

    # Preload the ScalarE Exp table off the critical path.
    nc.gpsimd.memset(warm, 0.0)
    nc.scalar.activation(out=warm[:, 1:2], in_=warm[:, 0:1], func=AF.Exp)
    nc.gpsimd.memset(ones, 1.0)

    # ---- Wave-1 DMAs (sync triggers, first in queue order): everything
    # the group-0 chain and first matmuls need. ----
    for k in range(2):
        for h in range(4):
            nc.sync.dma_start(
                out=ptk[k][:, h * 512 : (h + 1) * 512],
                in_=pt[k * 128 : (k + 1) * 128, h * 512 : (h + 1) * 512],
            )
    for k in range(2):
        for h in range(2):
            nc.sync.dma_start(
                out=pblkt_t[k][:, h * 512 : (h + 1) * 512],
                in_=pblkt[k * 128 : (k + 1) * 128, h * 512 : (h + 1) * 512],
            )
    pbo = pblk.rearrange("(u p) d -> p u d", p=128, u=UO)
    pblk_il = pin.tile([128, UO * D], BF16, name="pblk_il",
                       tag="pblk_il", bufs=1)
    for h in range(2):
        nc.sync.dma_start(
            out=pblk_il.rearrange("p (u d) -> p u d", u=UO)[:, h * 4 : (h + 1) * 4, :],
            in_=pbo[:, h * 4 : (h + 1) * 4, :],
        )

    # ---- Wave-2 DMAs (gpsimd triggers): P^T bulk for groups 1-3. ----
    for g in range(1, N_GROUPS):
        for k in range(2):
            for h in range(2):
                sl = slice(g * GROUP + h * 1024, g * GROUP + (h + 1) * 1024)
                nc.gpsimd.dma_start(out=ptk[k][:, sl], in_=pt[k * 128 : (k + 1) * 128, sl])

    # ---- Own-block row norms -> per-partition ACT scale 2/n_i.
    # Row i = 128u + p -> partition p, slot u, so scv[:, m] lines up with
    # the psum partition dim of row-tile m. ----
    sq_o = work.tile([128, UO * D], BF16, name="sq_o", tag="sq_o", bufs=1)
    nc.vector.tensor_mul(sq_o, pblk_il, pblk_il)
    with nc.allow_low_precision(reason="bf16 norm^2 partials, 0.4% ok"):
        nc.vector.tensor_reduce(
            nblk, sq_o.rearrange("p (u d) -> p u d", u=UO),
            axis=mybir.AxisListType.X, op=ALU.add,
        )
    nc.vector.tensor_copy(nblk_f, nblk)
    _newton_rsqrt(nc, work, nblk_f, nblk_f, tag="own")
    nc.vector.tensor_scalar(
        out=scv, in0=nblk_f, scalar1=2.0, scalar2=None, op0=ALU.mult,
    )

    # ---- Global column norms per 2048-column group: squares on DVE,
    # partition-reduce on TensorE (ones stationary), compact via
    # PSUM->SBUF copy + SBUF->SBUF reshape DMA: nsq[p, k] = |p_(64p+k)|^2.
    # ----
    def normalize_group(g):
        gofs = g * GROUP
        for k in range(2):
            nc.vector.tensor_mul(
                sqt[k], ptk[k][:, gofs : gofs + GROUP],
                ptk[k][:, gofs : gofs + GROUP],
            )
        nps = psum_pool.tile([128, GROUP], F32, name="nps", tag="ps")
        for c4 in range(N_PER_GROUP):
            for k in range(2):
                nc.tensor.matmul(
                    nps[:, c4 * N_COLS : (c4 + 1) * N_COLS],
                    ones,
                    sqt[k][:, c4 * N_COLS : (c4 + 1) * N_COLS],
                    start=(k == 0),
                    stop=(k == 1),
                )
        tmp = work.tile([1, GROUP], F32, name="tmp", tag="nrow", bufs=2)
        nc.vector.tensor_copy(tmp, nps[0:1, :])
        fsl = slice(64 * g, 64 * g + 64)
        nc.sync.dma_start(
            out=dram_nsq[g * GROUP : (g + 1) * GROUP], in_=tmp[0:1, :]
        )
        nc.sync.dma_start(
            out=nsq[:, fsl],
            in_=dram_nsq[g * GROUP : (g + 1) * GROUP].rearrange(
                "(p k) -> p k", p=32
            ),
        )
        _newton_rsqrt(nc, work, nsq[:, fsl], nsq[:, fsl], tag=f"g{g}")
        nc.vector.tensor_copy(rn_b16[:, fsl], nsq[:, fsl])
        nc.sync.dma_start(
            out=dram_rn[gofs : gofs + GROUP].rearrange("(p k) -> p k", p=32),
            in_=rn_b16[:, fsl],
        )
        rnb = work.tile([128, GROUP], BF16, name="rnb", tag="rnb", bufs=2)
        for h in range(4):
            nc.sync.dma_start(
                out=rnb[:, h * 512 : (h + 1) * 512],
                in_=dram_rn[
                    gofs + h * 512 : gofs + (h + 1) * 512
                ].partition_broadcast(128),
            )
        for k in range(2):
            for h in range(2):
                hsl = slice(gofs + h * 1024, gofs + (h + 1) * 1024)
                nc.vector.tensor_mul(
                    qtk[k][:, hsl],
                    ptk[k][:, hsl],
                    rnb[:, h * 1024 : (h + 1) * 1024],
                )

    normalize_group(0)

    # ---- Main loop: S-block matmuls + fused exp/row-sum.  Group g+1 norm
    # prep is emitted before group g's body (PSUM rotation stays ahead)
    # and fenced so it cannot invade the group-0 critical chain. ----
    for g in range(N_GROUPS):
        if g + 1 < N_GROUPS:
            with tc.tile_wait_until(0.009 + 0.017 * g):
                normalize_group(g + 1)
        for m in range(M_TILES):
            ps = psum_pool.tile([128, GROUP], F32, name="ps", tag="ps")
            for n4 in range(N_PER_GROUP):
                col = g * GROUP + n4 * N_COLS
                for k in range(2):
                    nc.tensor.matmul(
                        ps[:, n4 * N_COLS : (n4 + 1) * N_COLS],
                        pblkt_t[k][:, m * 128 : (m + 1) * 128],
                        qtk[k][:, col : col + N_COLS],
                        start=(k == 0),
                        stop=(k == 1),
                    )
            esc = epool.tile([128, GROUP], BF16, name="esc")
            nc.scalar.activation(
                out=esc,
                in_=ps,
                func=AF.Exp,
                scale=scv[:, m : m + 1],
                accum_out=sums[:, g * M_TILES + m : g * M_TILES + m + 1],
            )

    # ---- Epilogue: rowsum over groups, drop diagonal, log, write out ----
    sums3 = sums.rearrange("p (g m) -> p m g", g=N_GROUPS)
    nc.vector.tensor_reduce(rowsum, sums3, axis=mybir.AxisListType.X, op=ALU.add)
    nc.vector.tensor_scalar(
        out=lse, in0=rowsum, scalar1=E2, scalar2=None, op0=ALU.subtract,
    )
    nc.scalar.activation(out=lse, in_=lse, func=AF.Ln)
    nc.sync.dma_start(out=lse_out, in_=lse)

    for p in (psum_pool, epool, dram, work, pin, persist):
        p.release()


_BUILT = None


def _build():
    global _BUILT
    if _BUILT is None:
        nc = bacc.Bacc("TRN2", target_bir_lowering=False, debug=False,
                       num_devices=N_CORES)
        pt = nc.dram_tensor("pt", [D, B], BF16, kind="ExternalInput").ap()
        pblkt = nc.dram_tensor("pblkt", [D, BLK], BF16, kind="ExternalInput").ap()
        pblk = nc.dram_tensor("pblk", [BLK, D], BF16, kind="ExternalInput").ap()
        lse_out = nc.dram_tensor("lse_out", [128, M_TILES], F32,
                                 kind="ExternalOutput").ap()
        with tile.TileContext(nc) as tc:
            _emit(tc, pt, pblkt, pblk, lse_out)
        nc.finalize()
        _BUILT = nc
    return _BUILT


def run_on_hw(P, **spmd_kwargs):
    import jax.numpy as jnp

    nc = _build()
    P_b = np.asarray(jnp.asarray(P, dtype=jnp.bfloat16))
    PT_b = np.ascontiguousarray(P_b.T)
    in_maps = [
        {
            "pt": PT_b,
            "pblkt": np.ascontiguousarray(PT_b[:, c * BLK : (c + 1) * BLK]),
            "pblk": np.ascontiguousarray(P_b[c * BLK : (c + 1) * BLK]),
        }
        for c in range(N_CORES)
    ]
    return bass_utils.run_bass_kernel_spmd(
        nc, in_maps, core_ids=list(range(N_CORES)), **spmd_kwargs
    )


def kernel(embedding1, embedding2, projection1, projection2):
    import jax.numpy as jnp

    # embeddings are unused by the reference computation
    P = np.ascontiguousarray(
        np.concatenate([projection1, projection2], axis=0), dtype=np.float32
    )
    res = run_on_hw(P)
    # reassemble per-row lse: core c, tile column m, partition p ->
    # global row c*1024 + m*128 + p
    lse_rows = np.empty(B, np.float32)
    for c in range(N_CORES):
        arr = np.asarray(res.results[c]["lse_out"])  # [128, M_TILES]
        lse_rows[c * BLK : (c + 1) * BLK] = arr.T.reshape(-1)
    # Reference fp32 semantics: logp_ii = f32(-2e9 - lse_i) (== -2e9 for
    # any |lse| < 128), then loss = -mean(logp) with the platform's XLA
    # fp32 reduction -- reproduce it bit-for-bit.
    logp = (np.float32(-2.0e9) - lse_rows).astype(np.float32)
    loss = -jnp.mean(jnp.asarray(logp))
    return np.asarray(loss)
